# revision 1
# baseline (speedup 1.0000x reference)
"""Trainium2 kernel for nn_LAM_Module_19052474925494.

Reference computation (B,N,C,H,W = 16,10,128,48,48):
  q = k = x.reshape(B,N,D), D = C*H*W = 294912
  s0 = (1-pd)*k[n] + pd*k[n+1]        (indices mod N)
  s1 = ld*((1-pd)*k[n+1] + pd*k[n+2])
  logits = [q.s0, q.s1]; attn = softmax(logits); out = attn0*s0 + attn1*s1
  feat = out.reshape(B, N*C, H, W)
  result = conv1x1(conv_w, feat) + conv_b + x.reshape(B, N*C, H, W)

Key numeric fact exploited: logit0 - logit1 = 0.5*||x_n||^2 + 0.25*(q.k1) -
0.25*(q.k2) ~ 147000 >> 88 for iid N(0,1) inputs of this size, so the fp32
softmax saturates *exactly* to attn = [1, 0] (exp(-1.4e5) underflows to 0).
Hence feat_n = (1-pd_n)*x_n + pd_n*x_{n+1}, which is linear in x and can be
folded into the conv weights host-side:

  result[b] = (W_eff + I) @ X_b + bias,  X_b = x[b] as [N*C, H*W]
  W_eff[:, m*C:(m+1)*C] = (1-pd[m])*W[:, m*C:(m+1)*C] + pd[m-1]*W[:, (m-1)*C:...]

A host-side guard computes the actual logit gaps (3 dot products per (b,n),
one cheap pass over x) and only uses the folded form when every gap > 25
(a1 < 1.4e-11, far below fp16 matmul noise). Otherwise it falls back to
materializing feat with the true attention weights on the host and runs the
SAME device kernel with unfused weights (residual added back on host).

The device kernel is a single [1280x1280] @ [1280, 2304] matmul per batch
item (fp16 inputs, fp32 PSUM accumulation), data-parallel over batch:
2 batch items per NeuronCore across 8 cores. No collectives needed.
Measured: ~213 us HW exec on 8 cores, scale-relative absmax err ~6e-4
(PE streaming floor for this shape is ~192 us; the rest is engine preamble
+ the Tile-framework exit barrier).
"""

import numpy as np

B, N, C, H, W = 16, 10, 128, 48, 48
NCh = N * C   # 1280 channels
HW = H * W    # 2304 spatial
NCORES = 8
BB = B // NCORES  # batch items per core

# Tunables (test.py may override before first kernel() call)
IN_DTYPE = "f16"  # one of: f32r, bf16, f16, f32
NT_SIZE = 512
X_BUFS = 30
OB_GROUP = 1
OUT_BUFS = 16
WARMUP_MMS = 12  # dependency-free dummy matmuls to bridge + warm the PE at start
FIRST_DMA_ENGINE = "sync"  # engine issuing the first wt0/x0 loads
SPLIT_FIRST_DMA = False  # split first-stripe chunk DMAs into 2 for latency
F32R_DRAM = False  # declare xs/wt DRAM as float32r -> plain sync DMA, no cast
TRACE = False
TRACE_CORES = None  # e.g. list(range(8)) to profile every core
LAST_RESULT = None  # BassKernelResults of the last run (for profiling)

# Sub-batches: (batch item, col start, col width, ob group size). Each loads
# its own 10 X chunks over [col0, col0+cw); X_BUFS >= 20 lets the next
# sub-batch prefetch fully during compute. fp32r needs moving dim >= 256 for
# full PE rate, so widths decompose into 512/256 tiles.
# The first sub-batch is a narrow 512-col stripe swept kb-outer across 8
# output blocks at once, so the PE has ~1.7us of work per arriving 0.7us
# chunk DMA right from kernel start.
SUBS = [
    (0, 0, 512, 8),
    (0, 512, 1024, 1),
    (0, 1536, 768, 1),
    (1, 0, 1024, 1),
    (1, 1024, 1024, 1),
    (1, 2048, 256, 4),
]

_cache = {}


def _build_nc():
    import concourse.bacc as bacc
    import concourse.mybir as mybir
    from concourse.tile import TileContext

    f32 = mybir.dt.float32
    if IN_DTYPE == "bf16":
        in_dt = mybir.dt.bfloat16
    elif IN_DTYPE == "f16":
        in_dt = mybir.dt.float16
    elif IN_DTYPE == "f32r" and F32R_DRAM:
        in_dt = mybir.dt.float32r
    else:
        in_dt = f32
    nc = bacc.Bacc(None, target_bir_lowering=False, debug=False)
    xs = nc.dram_tensor("xs", [BB, NCh, HW], in_dt, kind="ExternalInput")
    wt = nc.dram_tensor("wt", [NCh, NCh], in_dt, kind="ExternalInput")
    bias = nc.dram_tensor("bias", [C, N], f32, kind="ExternalInput")
    out = nc.dram_tensor("out", [BB, NCh, HW], f32, kind="ExternalOutput")

    def tiles_of(col0, cw):
        # Decompose into tiles of <= NT_SIZE, all >= 256 wide (fp32r full-rate
        # needs moving dim >= 256): 896 -> 512+384, 768 -> 512+256, etc.
        out, c = [], col0
        rem = cw
        while rem > 0:
            w = min(NT_SIZE, rem)
            if rem - w != 0 and rem - w < 256:
                w = rem - 256
            out.append((c, w))
            c += w
            rem -= w
        return out

    max_rest = max(cw for si, (_, _, cw, _) in enumerate(SUBS) if si > 0)

    with TileContext(nc) as tc:
        with (
            tc.tile_pool(name="wtp", bufs=1) as wt_pool,
            tc.tile_pool(name="biasp", bufs=1) as bias_pool,
            tc.tile_pool(name="xp", bufs=X_BUFS) as x_pool,
            tc.tile_pool(name="psp", bufs=8, space="PSUM") as psum_pool,
            tc.tile_pool(name="op", bufs=OUT_BUFS) as out_pool,
        ):
            if IN_DTYPE == "bf16":
                mm_dt, mm_dma = mybir.dt.bfloat16, nc.sync
            elif IN_DTYPE == "f16":
                mm_dt, mm_dma = mybir.dt.float16, nc.sync
            elif IN_DTYPE == "f32r":
                mm_dt = mybir.dt.float32r
                mm_dma = nc.sync if F32R_DRAM else nc.gpsimd
            else:
                mm_dt, mm_dma = f32, nc.sync
            bias_sb = bias_pool.tile([C, N], f32, name="bias_sb")
            nc.sync.dma_start(out=bias_sb[:], in_=bias[:])

            if WARMUP_MMS:
                # PE warm-up: zero-dependency matmuls on a memset scratch tile
                # keep the PE busy (and the HAM clock-gate warm) while engine
                # preambles finish and the first real chunks stream in.
                wsc = bias_pool.tile([C, 512], mm_dt, name="warm_sc")
                nc.gpsimd.memset(wsc[:], 0.0)
                wps = psum_pool.tile([C, NT_SIZE], f32, tag="ps", name="warm_ps")
                for wi in range(WARMUP_MMS):
                    nc.tensor.matmul(
                        wps[:], wsc[:, :C], wsc[:], start=True, stop=True
                    )

            wt_sb = [None] * N

            def load_wt(kb, eng=None):
                t = wt_pool.tile([C, NCh], mm_dt, tag=f"wt{kb}", name=f"wt_sb{kb}")
                (eng or mm_dma).dma_start(out=t[:], in_=wt[kb * C : (kb + 1) * C, :])
                wt_sb[kb] = t

            x_tiles = {}

            def load_x(si, kb, eng=None):
                bi, col0, cw, _ = SUBS[si]
                if si == 0:
                    t = x_pool.tile(
                        [C, cw], mm_dt, tag="x0", bufs=N, name=f"x_{si}_{kb}"
                    )
                else:
                    t = x_pool.tile(
                        [C, max_rest], mm_dt, tag="x", name=f"x_{si}_{kb}"
                    )
                if si == 0 and SPLIT_FIRST_DMA:
                    hw2 = cw // 2
                    mm_dma.dma_start(
                        out=t[:, :hw2],
                        in_=xs[bi, kb * C : (kb + 1) * C, col0 : col0 + hw2],
                    )
                    mm_dma.dma_start(
                        out=t[:, hw2:cw],
                        in_=xs[bi, kb * C : (kb + 1) * C, col0 + hw2 : col0 + cw],
                    )
                else:
                    (eng or mm_dma).dma_start(
                        out=t[:, :cw],
                        in_=xs[bi, kb * C : (kb + 1) * C, col0 : col0 + cw],
                    )
                x_tiles[(si, kb)] = t

            # Interleave weight-chunk and first-sub-batch X loads so the PE
            # can start accumulating as soon as wt[0]+x[0] land.
            first_eng = {"sync": nc.sync, "vector": nc.vector, "scalar": nc.scalar}[
                FIRST_DMA_ENGINE
            ]
            for kb in range(N):
                eng = first_eng if kb < 2 and FIRST_DMA_ENGINE != "sync" else None
                load_x(0, kb, eng)
                load_wt(kb, eng)

            for si, (bi, col0, cw_sub, obg) in enumerate(SUBS):
                half = tiles_of(col0, cw_sub)
                if si + 1 < len(SUBS):
                    for kb in range(N):
                        load_x(si + 1, kb)
                for og in range(0, N, obg):
                    obs = list(range(og, min(og + obg, N)))
                    psums = {
                        (ob, ti): psum_pool.tile(
                            [C, NT_SIZE], f32, tag="ps", name=f"ps_{si}_{ob}_{ti}"
                        )
                        for ob in obs
                        for ti in range(len(half))
                    }
                    for kb in range(N):
                        xt = x_tiles[(si, kb)]
                        for ob in obs:
                            lhs = wt_sb[kb][:, ob * C : (ob + 1) * C]
                            for ti, (c0, cw) in enumerate(half):
                                rhs = xt[:, c0 - col0 : c0 - col0 + cw]
                                nc.tensor.matmul(
                                    psums[(ob, ti)][:, :cw], lhs, rhs,
                                    start=(kb == 0), stop=(kb == N - 1),
                                )
                    for ob in obs:
                        for ti, (c0, cw) in enumerate(half):
                            osb = out_pool.tile(
                                [C, NT_SIZE], f32, tag="o", name=f"o_{si}_{ob}_{ti}"
                            )
                            nc.vector.tensor_scalar_add(
                                osb[:, :cw], psums[(ob, ti)][:, :cw],
                                bias_sb[:, ob : ob + 1],
                            )
                            nc.sync.dma_start(
                                out=out[bi, ob * C : (ob + 1) * C, c0 : c0 + cw],
                                in_=osb[:, :cw],
                            )
    nc.finalize()
    return nc


def kernel(x, pos_dec, length_dec, conv_w, conv_b):
    global LAST_RESULT
    from concourse.bass_utils import run_bass_kernel_spmd

    pd = np.asarray(pos_dec, dtype=np.float32)
    ld = np.asarray(length_dec, dtype=np.float32)
    Wm = np.asarray(conv_w, dtype=np.float32)
    x = np.asarray(x, dtype=np.float32).reshape(B, N, C * H * W)

    # Guard: verify the 2-way softmax saturates to [1, 0] for this input.
    # logit0 - logit1 = (1-pd)*g0 + pd*g1 - ld*((1-pd)*g1 + pd*g2) with
    # g_j = <x_n, x_{n+j mod N}>; for iid N(0,1) data g0 ~ 294912 dominates.
    g0 = np.einsum("bnd,bnd->bn", x, x)
    x1 = np.roll(x, -1, axis=1)
    g1 = np.einsum("bnd,bnd->bn", x, x1)
    g2 = np.einsum("bnd,bnd->bn", x, np.roll(x, -2, axis=1))
    l0 = (1.0 - pd) * g0 + pd * g1
    l1 = ld * ((1.0 - pd) * g1 + pd * g2)
    saturated = bool((l0 - l1).min() > 25.0)

    if saturated:
        # attn == [1, 0] exactly in fp32 -> feat_n = (1-pd_n) x_n + pd_n x_{n+1};
        # fold interpolation + residual identity into the weights.
        W_eff = np.empty_like(Wm)
        for m in range(N):
            pm = (m - 1) % N
            W_eff[:, m * C : (m + 1) * C] = \
                (1.0 - pd[m]) * Wm[:, m * C : (m + 1) * C] + \
                pd[pm] * Wm[:, pm * C : (pm + 1) * C]
        idx = np.arange(NCh)
        W_eff[idx, idx] += 1.0
        feed = x
    else:
        # General path: materialize feat with the true attention weights on
        # the host; run the same device kernel with the plain conv weights
        # and add the residual back afterwards.
        gap = l1 - l0
        a1 = 1.0 / (1.0 + np.exp(np.clip(-gap, -87.0, 87.0)))
        a0 = 1.0 - a1
        c0 = (a0 * (1.0 - pd))[:, :, None]
        c1 = (a0 * pd + a1 * ld * (1.0 - pd))[:, :, None]
        c2 = (a1 * ld * pd)[:, :, None]
        feed = c0 * x + c1 * x1 + c2 * np.roll(x, -2, axis=1)
        W_eff = Wm

    in_np = np.float32
    if IN_DTYPE == "bf16":
        import ml_dtypes

        in_np = ml_dtypes.bfloat16
    elif IN_DTYPE == "f16":
        in_np = np.float16
    feed = np.ascontiguousarray(feed.reshape(B, NCh, HW).astype(in_np))
    WT = np.ascontiguousarray(W_eff.T.astype(in_np))  # [c_in, o] for lhsT
    bias_t = np.ascontiguousarray(
        np.asarray(conv_b, dtype=np.float32).reshape(N, C).T
    )  # [C, N]: column ob = biases of output block ob

    if "nc" not in _cache:
        _cache["nc"] = _build_nc()
    nc = _cache["nc"]

    in_maps = [
        {"xs": feed[c * BB : (c + 1) * BB], "wt": WT, "bias": bias_t}
        for c in range(NCORES)
    ]
    res = None
    for attempt in range(3):
        try:
            res = run_bass_kernel_spmd(
                nc, in_maps, core_ids=list(range(NCORES)), trace=TRACE,
                trace_cores=TRACE_CORES,
            )
            break
        except Exception:
            # The PJRT/axon dispatch occasionally hits a transient
            # device-unrecoverable error; a retry re-initializes and succeeds.
            if attempt == 2:
                raise
            import time

            time.sleep(2.0)
    LAST_RESULT = res
    out = np.concatenate([res.results[c]["out"] for c in range(NCORES)], axis=0)
    if not saturated:
        out = out + x.reshape(B, NCh, HW)
    return out.reshape(B, NCh, H, W)



# revision 3
# speedup vs baseline: 1.2925x; 1.2925x over previous
"""Trainium2 kernel for nn_LAM_Module_19052474925494 (mixed fp8/fp16 matmul).

Reference computation (B,N,C,H,W = 16,10,128,48,48):
  q = k = x.reshape(B,N,D), D = C*H*W = 294912
  s0 = (1-pd)*k[n] + pd*k[n+1]        (indices mod N)
  s1 = ld*((1-pd)*k[n+1] + pd*k[n+2])
  logits = [q.s0, q.s1]; attn = softmax(logits); out = attn0*s0 + attn1*s1
  feat = out.reshape(B, N*C, H, W)
  result = conv1x1(conv_w, feat) + conv_b + x.reshape(B, N*C, H, W)

For this input distribution the 2-way softmax saturates exactly to [1, 0]
(logit gap ~1.5e5; guarded host-side), so feat is linear in x and folds into
the conv weights: result[b] = W_eff @ X_b + bias + X_b.  The residual X_b is
added on the HOST (not folded into W_eff): device output is just W_eff @ X_b.

Device kernel: per-core [1280 x 1280] @ [1280 x 2304] matmul for 2 batch
items (data-parallel over batch, 8 cores). Mixed precision along the
contraction dim to exploit the PE's fp8 DoubleRow mode (2 rows/cycle, 2x
fp16 rate): FP8_CHUNKS of the ten 128-channel K-chunks run as e4m3
DoubleRow pairs, the rest as fp16 at 1x. W is scaled by SW=64 and X by SX=8
host-side so e4m3 sees a healthy binade range; the PSUM result is descaled
by 1/(SW*SX) in the bias-add vector op. Simulated end-to-end rel-err
(FP8_CHUNKS=6): 1.72e-2 vs the 2e-2 gate; pure fp16 floor is 192us/core,
this config's PE floor is ~134us/core.
"""

import numpy as np

B, N, C, H, W = 16, 10, 128, 48, 48
NCh = N * C   # 1280 channels
HW = H * W    # 2304 spatial
NCORES = 8
BB = B // NCORES  # batch items per core

# Tunables (test.py may override before first kernel() call)
FP8_CHUNKS = 6        # even; this many 128-ch K-chunks run as e4m3 DoubleRow
SW = 64.0             # host-side weight scale before fp8/fp16 quantization
SX = 8.0              # host-side activation scale
OUT_F16 = True        # device writes f16 output (halves write traffic)
WARMUP_MMS = 12
OUT_BUFS = 16
PS_BUFS = 8
COLT = [(0, 512), (512, 512), (1024, 512), (1536, 512), (2048, 256)]
TRACE = False
TRACE_CORES = None
LAST_RESULT = None

NP8 = FP8_CHUNKS // 2
N16 = N - FP8_CHUNKS

_cache = {}


def _build_nc():
    import concourse.bacc as bacc
    import concourse.mybir as mybir
    from concourse.tile import TileContext

    global NP8, N16
    NP8 = FP8_CHUNKS // 2
    N16 = N - FP8_CHUNKS

    f32 = mybir.dt.float32
    f16 = mybir.dt.float16
    f8 = mybir.dt.float8e4
    DR = mybir.MatmulPerfMode.DoubleRow
    out_dt = f16 if OUT_F16 else f32

    nc = bacc.Bacc(None, target_bir_lowering=False, debug=False)
    xs8 = nc.dram_tensor("xs8", [BB, NP8, C, 2 * HW], f8, kind="ExternalInput")
    xs16 = nc.dram_tensor("xs16", [BB, max(N16, 1), C, HW], f16, kind="ExternalInput")
    wt8 = nc.dram_tensor("wt8", [NP8, C, 2 * NCh], f8, kind="ExternalInput")
    wt16 = nc.dram_tensor("wt16", [max(N16, 1), C, NCh], f16, kind="ExternalInput")
    bias = nc.dram_tensor("bias", [C, N], f32, kind="ExternalInput")
    out = nc.dram_tensor("out", [BB, NCh, HW], out_dt, kind="ExternalOutput")

    descale = 1.0 / (SW * SX)

    with TileContext(nc) as tc:
        with (
            tc.tile_pool(name="wtp", bufs=1) as wt_pool,
            tc.tile_pool(name="biasp", bufs=1) as bias_pool,
            tc.tile_pool(name="xp", bufs=1) as x_pool,
            tc.tile_pool(name="psp", bufs=PS_BUFS, space="PSUM") as psum_pool,
            tc.tile_pool(name="op", bufs=OUT_BUFS) as out_pool,
        ):
            bias_sb = bias_pool.tile([C, N], f32, name="bias_sb")
            nc.sync.dma_start(out=bias_sb[:], in_=bias[:])

            if WARMUP_MMS:
                # PE warm-up: zero-dependency DR matmuls keep the PE busy and
                # ramp the p-state while preambles finish and DMA streams in.
                wsc = bias_pool.tile([C, 2, 256], f8, name="warm_sc")
                nc.gpsimd.memset(wsc[:], 0.0)
                wps = psum_pool.tile([C, 512], f32, tag="ps", name="warm_ps")
                for wi in range(WARMUP_MMS):
                    nc.tensor.matmul(
                        wps[:, :256], wsc[:, :, :C], wsc[:],
                        start=True, stop=True, perf_mode=DR,
                    )

            wt8_sb, wt16_sb, x8_sb, x16_sb = {}, {}, {}, {}

            def load_wt(j):
                if j < NP8:
                    t = wt_pool.tile(
                        [C, 2, NCh], f8, tag=f"wt8_{j}", name=f"wt8_sb{j}"
                    )
                    for i in (0, 1):
                        nc.scalar.dma_start(
                            out=t[:, i, :], in_=wt8[j, :, i * NCh : (i + 1) * NCh]
                        )
                    wt8_sb[j] = t
                else:
                    k = j - NP8
                    t = wt_pool.tile([C, NCh], f16, tag=f"wt16_{k}", name=f"wt16_sb{k}")
                    nc.scalar.dma_start(out=t[:], in_=wt16[k, :, :])
                    wt16_sb[k] = t

            def alloc_x(it):
                for p in range(NP8):
                    x8_sb[(it, p)] = x_pool.tile(
                        [C, 2, HW], f8, tag=f"x8_{it}_{p}", name=f"x8_{it}_{p}"
                    )
                for k in range(N16):
                    x16_sb[(it, k)] = x_pool.tile(
                        [C, HW], f16, tag=f"x16_{it}_{k}", name=f"x16_{it}_{k}"
                    )

            def load_stripe(it, t_idx, chunks=None):
                c0, cw = COLT[t_idx]
                for j in chunks if chunks is not None else range(NP8 + N16):
                    if j < NP8:
                        t = x8_sb[(it, j)]
                        for i in (0, 1):
                            nc.sync.dma_start(
                                out=t[:, i, c0 : c0 + cw],
                                in_=xs8[it, j, :, i * HW + c0 : i * HW + c0 + cw],
                            )
                    else:
                        k = j - NP8
                        nc.sync.dma_start(
                            out=x16_sb[(it, k)][:, c0 : c0 + cw],
                            in_=xs16[it, k, :, c0 : c0 + cw],
                        )

            def mm(ps, it, j, ob, c0, cw, start, stop):
                if j < NP8:
                    nc.tensor.matmul(
                        ps[:, :cw],
                        wt8_sb[j][:, :, ob * C : (ob + 1) * C],
                        x8_sb[(it, j)][:, :, c0 : c0 + cw],
                        start=start, stop=stop, perf_mode=DR,
                    )
                else:
                    k = j - NP8
                    nc.tensor.matmul(
                        ps[:, :cw],
                        wt16_sb[k][:, ob * C : (ob + 1) * C],
                        x16_sb[(it, k)][:, c0 : c0 + cw],
                        start=start, stop=stop,
                    )

            nj = NP8 + N16

            def drain(ps, it, ob, c0, cw):
                osb = out_pool.tile([C, 512], out_dt, tag="o", name=f"o_{it}_{ob}_{c0}")
                nc.vector.tensor_scalar(
                    osb[:, :cw], ps[:, :cw], descale, bias_sb[:, ob : ob + 1],
                    mybir.AluOpType.mult, mybir.AluOpType.add,
                )
                nc.gpsimd.dma_start(
                    out=out[it, ob * C : (ob + 1) * C, c0 : c0 + cw], in_=osb[:, :cw]
                )

            def group(it, ob, c0, cw):
                ps = psum_pool.tile([C, 512], f32, tag="ps", name=f"ps_{it}_{ob}_{c0}")
                for j in range(nj):
                    mm(ps, it, j, ob, c0, cw, j == 0, j == nj - 1)
                drain(ps, it, ob, c0, cw)

            alloc_x(0)
            alloc_x(1)

            # Weights + item0 stripe0, interleaved chunk-wise so the first
            # kb-outer sweep can chase arrivals.
            for j in range(nj):
                load_wt(j)
                load_stripe(0, 0, [j])

            load_stripe(0, 1)

            # First sweep: item0 coltile0, kb-outer across 8 output blocks at
            # once -> each arriving (wt, x) chunk unlocks 8 matmuls.
            c0, cw = COLT[0]
            ps0 = {
                ob: psum_pool.tile([C, 512], f32, tag="ps", name=f"ps0_{ob}")
                for ob in range(8)
            }
            for j in range(nj):
                for ob in range(8):
                    mm(ps0[ob], 0, j, ob, c0, cw, j == 0, j == nj - 1)
            for ob in range(8):
                drain(ps0[ob], 0, ob, c0, cw)
            for ob in (8, 9):
                group(0, ob, c0, cw)

            # item0 coltiles 1..4, with item0/item1 stripe loads interleaved
            # ahead of need.
            preload = [(0, 2), (1, 0), (0, 3), (1, 1), (0, 4), (1, 2), (1, 3), (1, 4)]
            pi = 0
            for t in range(1, len(COLT)):
                for _ in range(2):
                    if pi < len(preload):
                        load_stripe(*preload[pi])
                        pi += 1
                c0, cw = COLT[t]
                for ob in range(N):
                    group(0, ob, c0, cw)
            while pi < len(preload):
                load_stripe(*preload[pi])
                pi += 1
            for t in range(len(COLT)):
                c0, cw = COLT[t]
                for ob in range(N):
                    group(1, ob, c0, cw)
    nc.finalize()
    return nc


def kernel(x, pos_dec, length_dec, conv_w, conv_b):
    global LAST_RESULT
    import ml_dtypes
    from concourse.bass_utils import run_bass_kernel_spmd

    e4 = ml_dtypes.float8_e4m3
    pd = np.asarray(pos_dec, dtype=np.float32)
    ld = np.asarray(length_dec, dtype=np.float32)
    Wm = np.asarray(conv_w, dtype=np.float32)
    x = np.asarray(x, dtype=np.float32).reshape(B, N, C * H * W)

    # Guard: verify the 2-way softmax saturates to [1, 0] for this input.
    g0 = np.einsum("bnd,bnd->bn", x, x)
    x1 = np.roll(x, -1, axis=1)
    g1 = np.einsum("bnd,bnd->bn", x, x1)
    g2 = np.einsum("bnd,bnd->bn", x, np.roll(x, -2, axis=1))
    l0 = (1.0 - pd) * g0 + pd * g1
    l1 = ld * ((1.0 - pd) * g1 + pd * g2)
    saturated = bool((l0 - l1).min() > 25.0)

    if saturated:
        # attn == [1, 0] exactly in fp32 -> feat_n = (1-pd_n) x_n + pd_n x_{n+1};
        # fold the interpolation into the weights (residual stays on host).
        W_eff = np.empty_like(Wm)
        for m in range(N):
            pm = (m - 1) % N
            W_eff[:, m * C : (m + 1) * C] = \
                (1.0 - pd[m]) * Wm[:, m * C : (m + 1) * C] + \
                pd[pm] * Wm[:, pm * C : (pm + 1) * C]
        feed = x
    else:
        # General path: materialize feat with the true attention weights.
        gap = l1 - l0
        a1 = 1.0 / (1.0 + np.exp(np.clip(-gap, -87.0, 87.0)))
        a0 = 1.0 - a1
        c0 = (a0 * (1.0 - pd))[:, :, None]
        c1 = (a0 * pd + a1 * ld * (1.0 - pd))[:, :, None]
        c2 = (a1 * ld * pd)[:, :, None]
        feed = c0 * x + c1 * x1 + c2 * np.roll(x, -2, axis=1)
        W_eff = Wm

    nsplit = FP8_CHUNKS * C
    feed = feed.reshape(B, NCh, HW)
    # X8: [B, NP8, C, 2*HW], partition row c holds channels (256p+c, 256p+128+c)
    f8part = (feed[:, :nsplit, :] * SX).reshape(B, NP8, 2, C, HW)
    X8 = np.ascontiguousarray(f8part.transpose(0, 1, 3, 2, 4)).reshape(
        B, NP8, C, 2 * HW
    )
    X8 = np.clip(X8, -240, 240).astype(e4)
    X16 = np.ascontiguousarray(
        (feed[:, nsplit:, :] * SX).reshape(B, max(N16, 1), C, HW)
    ).astype(np.float16)

    WT = (W_eff * SW).T  # [c_in, o]
    W8 = WT[:nsplit].reshape(NP8, 2, C, NCh).transpose(0, 2, 1, 3)
    W8 = np.clip(np.ascontiguousarray(W8).reshape(NP8, C, 2 * NCh), -240, 240).astype(e4)
    W16 = np.ascontiguousarray(WT[nsplit:].reshape(max(N16, 1), C, NCh)).astype(
        np.float16
    )
    bias_t = np.ascontiguousarray(
        np.asarray(conv_b, dtype=np.float32).reshape(N, C).T
    )  # [C, N]: column ob = biases of output block ob

    if "nc" not in _cache:
        _cache["nc"] = _build_nc()
    nc = _cache["nc"]

    in_maps = [
        {
            "xs8": X8[c * BB : (c + 1) * BB],
            "xs16": X16[c * BB : (c + 1) * BB],
            "wt8": W8,
            "wt16": W16,
            "bias": bias_t,
        }
        for c in range(NCORES)
    ]
    res = None
    for attempt in range(3):
        try:
            res = run_bass_kernel_spmd(
                nc, in_maps, core_ids=list(range(NCORES)), trace=TRACE,
                trace_cores=TRACE_CORES,
            )
            break
        except Exception:
            # The PJRT/axon dispatch occasionally hits a transient
            # device-unrecoverable error; a retry re-initializes and succeeds.
            if attempt == 2:
                raise
            import time

            time.sleep(2.0)
    LAST_RESULT = res
    out = np.concatenate(
        [np.asarray(res.results[c]["out"]) for c in range(NCORES)], axis=0
    ).astype(np.float32)
    out = out + feed if saturated else out + x.reshape(B, NCh, HW)
    # residual is x in both paths; in the saturated path feed IS x.
    return out.reshape(B, NCh, H, W)


# revision 8
# speedup vs baseline: 1.3053x; 1.0099x over previous
"""Trainium2 kernel for nn_LAM_Module_19052474925494 (mixed fp8/fp16 matmul).

Reference computation (B,N,C,H,W = 16,10,128,48,48):
  q = k = x.reshape(B,N,D), D = C*H*W = 294912
  s0 = (1-pd)*k[n] + pd*k[n+1]        (indices mod N)
  s1 = ld*((1-pd)*k[n+1] + pd*k[n+2])
  logits = [q.s0, q.s1]; attn = softmax(logits); out = attn0*s0 + attn1*s1
  feat = out.reshape(B, N*C, H, W)
  result = conv1x1(conv_w, feat) + conv_b + x.reshape(B, N*C, H, W)

For this input distribution the 2-way softmax saturates exactly to [1, 0]
(logit gap ~1.5e5; guarded host-side), so feat is linear in x and folds into
the conv weights: result[b] = W_eff @ X_b + bias + X_b.  The residual X_b is
added on the HOST (not folded into W_eff): device output is just W_eff @ X_b.

Device kernel: per-core [1280 x 1280] @ [1280 x 2304] matmul for 2 batch
items (data-parallel over batch, 8 cores). Mixed precision along the
contraction dim to exploit the PE's fp8 DoubleRow mode (2 rows/cycle, 2x
fp16 rate): FP8_CHUNKS of the ten 128-channel K-chunks run as e4m3
DoubleRow pairs, the rest as fp16 at 1x. W is scaled by SW=64 and X by SX=8
host-side so e4m3 sees a healthy binade range; the PSUM result is descaled
by 1/(SW*SX) in the bias-add vector op. Simulated end-to-end rel-err
(FP8_CHUNKS=6): 1.72e-2 vs the 2e-2 gate; pure fp16 floor is 192us/core,
this config's PE floor is ~134us/core.
"""

import numpy as np

B, N, C, H, W = 16, 10, 128, 48, 48
NCh = N * C   # 1280 channels
HW = H * W    # 2304 spatial
NCORES = 8
BB = B // NCORES  # batch items per core

# Tunables (test.py may override before first kernel() call)
FP8_CHUNKS = 6        # even; this many 128-ch K-chunks run as e4m3 DoubleRow
SW = 64.0             # host-side weight scale before fp8/fp16 quantization
SX = 8.0              # host-side activation scale
OUT_F16 = True        # device writes f16 output (halves write traffic)
WARMUP_MMS = 20
OUT_BUFS = 16
PS_BUFS = 8
COLT = [(0, 512), (512, 512), (1024, 512), (1536, 512), (2048, 256)]
NT = len(COLT)
TRACE = False
TRACE_CORES = None
LAST_RESULT = None

NP8 = FP8_CHUNKS // 2
N16 = N - FP8_CHUNKS

_cache = {}


def _build_nc():
    import concourse.bacc as bacc
    import concourse.mybir as mybir
    from concourse.tile import TileContext

    global NP8, N16
    NP8 = FP8_CHUNKS // 2
    N16 = N - FP8_CHUNKS

    f32 = mybir.dt.float32
    f16 = mybir.dt.float16
    f8 = mybir.dt.float8e4
    DR = mybir.MatmulPerfMode.DoubleRow
    out_dt = f16 if OUT_F16 else f32

    nc = bacc.Bacc(None, target_bir_lowering=False, debug=False)
    xs8 = nc.dram_tensor("xs8", [BB, NP8, C, 2 * HW], f8, kind="ExternalInput")
    xs16 = nc.dram_tensor("xs16", [BB, max(N16, 1), C, HW], f16, kind="ExternalInput")
    wt8 = nc.dram_tensor("wt8", [NP8, C, 2 * NCh], f8, kind="ExternalInput")
    wt16 = nc.dram_tensor("wt16", [max(N16, 1), C, NCh], f16, kind="ExternalInput")
    bias = nc.dram_tensor("bias", [C, N], f32, kind="ExternalInput")
    # Output is written as per-group contiguous [C, 512] slabs (ob-major,
    # coltile-minor) so each drain DMA is a single sequential HBM burst; the
    # host reassembles. Last coltile uses only 256 of its 512 slot columns.
    out = nc.dram_tensor("out", [BB, N, NT, C, 512], out_dt, kind="ExternalOutput")

    descale = 1.0 / (SW * SX)

    with TileContext(nc) as tc:
        with (
            tc.tile_pool(name="wtp", bufs=1) as wt_pool,
            tc.tile_pool(name="biasp", bufs=1) as bias_pool,
            tc.tile_pool(name="xp", bufs=1) as x_pool,
            tc.tile_pool(name="psp", bufs=PS_BUFS, space="PSUM") as psum_pool,
            tc.tile_pool(name="op", bufs=OUT_BUFS) as out_pool,
        ):
            bias_sb = bias_pool.tile([C, N], f32, name="bias_sb")
            nc.sync.dma_start(out=bias_sb[:], in_=bias[:])

            if WARMUP_MMS:
                # PE warm-up: zero-dependency DR matmuls keep the PE busy and
                # ramp the p-state while preambles finish and DMA streams in.
                wsc = bias_pool.tile([C, 2, 256], f8, name="warm_sc")
                nc.gpsimd.memset(wsc[:], 0.0)
                wps = psum_pool.tile([C, 512], f32, tag="ps", name="warm_ps")
                for wi in range(WARMUP_MMS):
                    nc.tensor.matmul(
                        wps[:, :256], wsc[:, :, :C], wsc[:],
                        start=True, stop=True, perf_mode=DR,
                    )

            wt8_sb, wt16_sb, x8_sb, x16_sb = {}, {}, {}, {}

            def load_wt(j):
                # fp8 pair weights on the scalar queue, fp16 weights on the
                # gpsimd queue: three queues stream the startup working set.
                if j < NP8:
                    t = wt_pool.tile(
                        [C, 2, NCh], f8, tag=f"wt8_{j}", name=f"wt8_sb{j}"
                    )
                    for i in (0, 1):
                        nc.scalar.dma_start(
                            out=t[:, i, :], in_=wt8[j, :, i * NCh : (i + 1) * NCh]
                        )
                    wt8_sb[j] = t
                else:
                    k = j - NP8
                    t = wt_pool.tile([C, NCh], f16, tag=f"wt16_{k}", name=f"wt16_sb{k}")
                    nc.gpsimd.dma_start(out=t[:], in_=wt16[k, :, :])
                    wt16_sb[k] = t

            def alloc_x(it):
                for p in range(NP8):
                    x8_sb[(it, p)] = x_pool.tile(
                        [C, 2, HW], f8, tag=f"x8_{it}_{p}", name=f"x8_{it}_{p}"
                    )
                for k in range(N16):
                    x16_sb[(it, k)] = x_pool.tile(
                        [C, HW], f16, tag=f"x16_{it}_{k}", name=f"x16_{it}_{k}"
                    )

            def load_stripe(it, t_idx, chunks=None):
                c0, cw = COLT[t_idx]
                for j in chunks if chunks is not None else range(NP8 + N16):
                    if j < NP8:
                        t = x8_sb[(it, j)]
                        for i in (0, 1):
                            nc.sync.dma_start(
                                out=t[:, i, c0 : c0 + cw],
                                in_=xs8[it, j, :, i * HW + c0 : i * HW + c0 + cw],
                            )
                    else:
                        k = j - NP8
                        nc.sync.dma_start(
                            out=x16_sb[(it, k)][:, c0 : c0 + cw],
                            in_=xs16[it, k, :, c0 : c0 + cw],
                        )

            def mm(ps, it, j, ob, c0, cw, start, stop):
                if j < NP8:
                    nc.tensor.matmul(
                        ps[:, :cw],
                        wt8_sb[j][:, :, ob * C : (ob + 1) * C],
                        x8_sb[(it, j)][:, :, c0 : c0 + cw],
                        start=start, stop=stop, perf_mode=DR,
                    )
                else:
                    k = j - NP8
                    nc.tensor.matmul(
                        ps[:, :cw],
                        wt16_sb[k][:, ob * C : (ob + 1) * C],
                        x16_sb[(it, k)][:, c0 : c0 + cw],
                        start=start, stop=stop,
                    )

            nj = NP8 + N16
            tix = {c0: t for t, (c0, _) in enumerate(COLT)}
            drain_ct = [0]

            def drain(ps, it, ob, c0, cw):
                osb = out_pool.tile([C, 512], out_dt, tag="o", name=f"o_{it}_{ob}_{c0}")
                nc.vector.tensor_scalar(
                    osb[:, :cw], ps[:, :cw], descale, bias_sb[:, ob : ob + 1],
                    mybir.AluOpType.mult, mybir.AluOpType.add,
                )
                # Alternate output DMAs across two queues so the write stream
                # keeps up with compute.
                eng = nc.gpsimd if drain_ct[0] % 2 == 0 else nc.scalar
                drain_ct[0] += 1
                eng.dma_start(out=out[it, ob, tix[c0], :, :cw], in_=osb[:, :cw])

            def group(it, ob, c0, cw):
                ps = psum_pool.tile([C, 512], f32, tag="ps", name=f"ps_{it}_{ob}_{c0}")
                for j in range(nj):
                    mm(ps, it, j, ob, c0, cw, j == 0, j == nj - 1)
                drain(ps, it, ob, c0, cw)

            alloc_x(0)
            alloc_x(1)

            # Weights + item0 stripe0, interleaved chunk-wise so the first
            # kb-outer sweep can chase arrivals.
            for j in range(nj):
                load_wt(j)
                load_stripe(0, 0, [j])

            load_stripe(0, 1)

            # First sweep: item0 coltile0, kb-outer across 8 output blocks at
            # once -> each arriving (wt, x) chunk unlocks 8 matmuls.
            c0, cw = COLT[0]
            ps0 = {
                ob: psum_pool.tile([C, 512], f32, tag="ps", name=f"ps0_{ob}")
                for ob in range(8)
            }
            for j in range(nj):
                for ob in range(8):
                    mm(ps0[ob], 0, j, ob, c0, cw, j == 0, j == nj - 1)
            for ob in range(8):
                drain(ps0[ob], 0, ob, c0, cw)
            for ob in (8, 9):
                group(0, ob, c0, cw)

            # item0 coltiles 1..4, with item0/item1 stripe loads interleaved
            # ahead of need.
            preload = [(0, 2), (1, 0), (0, 3), (1, 1), (0, 4), (1, 2), (1, 3), (1, 4)]
            pi = 0
            for t in range(1, len(COLT)):
                for _ in range(2):
                    if pi < len(preload):
                        load_stripe(*preload[pi])
                        pi += 1
                c0, cw = COLT[t]
                for ob in range(N):
                    group(0, ob, c0, cw)
            while pi < len(preload):
                load_stripe(*preload[pi])
                pi += 1
            for t in range(len(COLT)):
                c0, cw = COLT[t]
                for ob in range(N):
                    group(1, ob, c0, cw)
    nc.finalize()
    return nc


def kernel(x, pos_dec, length_dec, conv_w, conv_b):
    global LAST_RESULT
    import ml_dtypes
    from concourse.bass_utils import run_bass_kernel_spmd

    e4 = ml_dtypes.float8_e4m3
    pd = np.asarray(pos_dec, dtype=np.float32)
    ld = np.asarray(length_dec, dtype=np.float32)
    Wm = np.asarray(conv_w, dtype=np.float32)
    x = np.asarray(x, dtype=np.float32).reshape(B, N, C * H * W)

    # Guard: verify the 2-way softmax saturates to [1, 0] for this input.
    g0 = np.einsum("bnd,bnd->bn", x, x)
    x1 = np.roll(x, -1, axis=1)
    g1 = np.einsum("bnd,bnd->bn", x, x1)
    g2 = np.einsum("bnd,bnd->bn", x, np.roll(x, -2, axis=1))
    l0 = (1.0 - pd) * g0 + pd * g1
    l1 = ld * ((1.0 - pd) * g1 + pd * g2)
    saturated = bool((l0 - l1).min() > 25.0)

    if saturated:
        # attn == [1, 0] exactly in fp32 -> feat_n = (1-pd_n) x_n + pd_n x_{n+1};
        # fold the interpolation into the weights (residual stays on host).
        W_eff = np.empty_like(Wm)
        for m in range(N):
            pm = (m - 1) % N
            W_eff[:, m * C : (m + 1) * C] = \
                (1.0 - pd[m]) * Wm[:, m * C : (m + 1) * C] + \
                pd[pm] * Wm[:, pm * C : (pm + 1) * C]
        feed = x
    else:
        # General path: materialize feat with the true attention weights.
        gap = l1 - l0
        a1 = 1.0 / (1.0 + np.exp(np.clip(-gap, -87.0, 87.0)))
        a0 = 1.0 - a1
        c0 = (a0 * (1.0 - pd))[:, :, None]
        c1 = (a0 * pd + a1 * ld * (1.0 - pd))[:, :, None]
        c2 = (a1 * ld * pd)[:, :, None]
        feed = c0 * x + c1 * x1 + c2 * np.roll(x, -2, axis=1)
        W_eff = Wm

    nsplit = FP8_CHUNKS * C
    feed = feed.reshape(B, NCh, HW)
    # X8: [B, NP8, C, 2*HW], partition row c holds channels (256p+c, 256p+128+c)
    f8part = (feed[:, :nsplit, :] * SX).reshape(B, NP8, 2, C, HW)
    X8 = np.ascontiguousarray(f8part.transpose(0, 1, 3, 2, 4)).reshape(
        B, NP8, C, 2 * HW
    )
    X8 = np.clip(X8, -240, 240).astype(e4)
    X16 = np.ascontiguousarray(
        (feed[:, nsplit:, :] * SX).reshape(B, max(N16, 1), C, HW)
    ).astype(np.float16)

    WT = (W_eff * SW).T  # [c_in, o]
    W8 = WT[:nsplit].reshape(NP8, 2, C, NCh).transpose(0, 2, 1, 3)
    W8 = np.clip(np.ascontiguousarray(W8).reshape(NP8, C, 2 * NCh), -240, 240).astype(e4)
    W16 = np.ascontiguousarray(WT[nsplit:].reshape(max(N16, 1), C, NCh)).astype(
        np.float16
    )
    bias_t = np.ascontiguousarray(
        np.asarray(conv_b, dtype=np.float32).reshape(N, C).T
    )  # [C, N]: column ob = biases of output block ob

    if "nc" not in _cache:
        _cache["nc"] = _build_nc()
    nc = _cache["nc"]

    in_maps = [
        {
            "xs8": X8[c * BB : (c + 1) * BB],
            "xs16": X16[c * BB : (c + 1) * BB],
            "wt8": W8,
            "wt16": W16,
            "bias": bias_t,
        }
        for c in range(NCORES)
    ]
    res = None
    for attempt in range(3):
        try:
            res = run_bass_kernel_spmd(
                nc, in_maps, core_ids=list(range(NCORES)), trace=TRACE,
                trace_cores=TRACE_CORES,
            )
            break
        except Exception:
            # The PJRT/axon dispatch occasionally hits a transient
            # device-unrecoverable error; a retry re-initializes and succeeds.
            if attempt == 2:
                raise
            import time

            time.sleep(2.0)
    LAST_RESULT = res
    slabs = np.concatenate(
        [np.asarray(res.results[c]["out"]) for c in range(NCORES)], axis=0
    )  # [B, N, NT, C, 512]
    out = np.empty((B, NCh, HW), np.float32)
    for t, (c0, cw) in enumerate(COLT):
        out[:, :, c0 : c0 + cw] = (
            slabs[:, :, t, :, :cw].reshape(B, NCh, cw).astype(np.float32)
        )
    out = out + feed if saturated else out + x.reshape(B, NCh, HW)
    # residual is x in both paths; in the saturated path feed IS x.
    return out.reshape(B, NCh, H, W)
